# revision 11
# baseline (speedup 1.0000x reference)
"""Trainium2 Bass kernel for DiffeomorphicLearner (gnn_message_passing).

Math (per step t, T=8 steps):
    sq_i  = ||z_i||^2
    K_ij  = exp((2 z_i.z_j - sq_i - sq_j) / (2 rho^2))
    v     = Z @ Aaff_t.T + b_t + K @ A_t
    Z    <- Z + DT * v

Distribution: row-parallel over N=4096 across 8 cores (512 rows each).
The per-step exchange of Z is done with direct core-to-core SBUF writes
(remote_dma_broadcast) instead of a collective AllGather: each core
broadcasts its fp8 payload [z^T (2ch x 512) | sq bits | const] to its 7
peers, landing in XOR-slot order (slot d holds data from core me^d, so
the receive address is compile-time constant in the SPMD program).
Per-slot arrival semaphores let the next step's S = Z_j.Z_i matmuls
consume slots stale-first (self slot 0 first, same-die next, cross-die
last), hiding the wire latency behind compute. A data-less notify
broadcast implements the WAR handshake for the double-buffered receive
buffers. All remote-arrival waits are attached to the consumer
instructions AFTER Tile scheduling (the single-core scheduling sim
cannot see cross-core semaphore increments).

Cross-die D2D lanes route with bit 1 of the tpb flipped (measured:
a send addressed (0,d) with d&4 lands on core s^d^2) — compensated by
addressing (0, d^2) for those slots.

Precision: all matmuls fp8 DoubleRow / fp32 PSUM. exp(-c sq_j) enters
as a per-partition ACT bias (bias_col); exp(-c sq_i) post-multiplies
vr. The A_t stream is XOR-permuted per core on the host so slot order
matches.
"""

import numpy as np
import ml_dtypes

import concourse.bass as bass
import concourse.tile as tile
from concourse import bacc, mybir
from concourse import bass_utils

BF16NP = ml_dtypes.bfloat16
F8NP = ml_dtypes.float8_e4m3

N_CORES = 8
N, D, T = 4096, 256, 8
RHO = 16.0
DT = 1.0 / T
CEXP = 1.0 / (2.0 * RHO * RHO)  # 1/512
SA = 64.0                      # fp8 scale for A (values are subnormal otherwise)
SAF = 64.0                     # fp8 scale for A_aff / b_aff

NLOC = N // N_CORES            # 512 rows per core
NJB = N // 128                 # 32 j-blocks of 128
NJB_LOC = NLOC // 128          # 4 j-blocks per slot
NPAIR = NJB // 2               # 16 block pairs (DoubleRow)
PAYC = 1040                    # payload cols: 1024 z + 8 sq + 1 cexp + 7 pad
PIPE_LAG = 2                   # pair skew between S/exp and K@A matmuls

F32 = mybir.dt.float32
BF16 = mybir.dt.bfloat16
F8 = mybir.dt.float8e4
DR = mybir.MatmulPerfMode.DoubleRow

_CACHED = {}


def _rdest(d):
    """XOR-relative dest for slot d, compensating the D2D bit-1 flip."""
    return (0, d ^ 2) if d & 4 else (0, d)


def _build():
    nc = bacc.Bacc("TRN2", target_bir_lowering=False, debug=False,
                   num_devices=N_CORES, num_swdge_queues=1)

    # ---- DRAM I/O -------------------------------------------------------
    zt_local0 = nc.dram_tensor("zt_local0", [D, NLOC], F32, kind="ExternalInput")
    zbp0 = nc.dram_tensor("zbp0", [128, PAYC], F8, kind="ExternalInput")
    recv0 = nc.dram_tensor("recv0", [128, 7 * PAYC], F8, kind="ExternalInput")
    e_row0 = nc.dram_tensor("e_row0", [1, NLOC], BF16, kind="ExternalInput")
    a_b = nc.dram_tensor("a_b", [T, N, D], F8, kind="ExternalInput")
    aaff_b = nc.dram_tensor("aaff_b", [T, D, D], F8, kind="ExternalInput")
    b_b = nc.dram_tensor("b_b", [T, 1, D], BF16, kind="ExternalInput")
    ones_col = nc.dram_tensor("ones_col", [128, 1], BF16, kind="ExternalInput")
    ones_row = nc.dram_tensor("ones_row", [1, NLOC], BF16, kind="ExternalInput")
    log_inv_sa = nc.dram_tensor("log_inv_sa", [1, 1], F32, kind="ExternalInput")
    cexp0 = nc.dram_tensor("cexp0", [128, 1], F32, kind="ExternalInput")
    out_zt = nc.dram_tensor("out_zt", [D, NLOC], F32, kind="ExternalOutput")

    EXP = mybir.ActivationFunctionType.Exp
    SQUARE = mybir.ActivationFunctionType.Square

    # Semaphores for the remote exchange. No explicit WAR handshake is
    # needed for the double-buffered receive buffers: a peer can only fire
    # exchange(t+2) (which overwrites my parity-p buffer) after its own
    # step t+1, which consumes MY exchange(t+1) (slot-sem gated), which I
    # only send after update(t) -- transitively after all my parity-p
    # reads of step t. The slot sems provide the back-pressure.
    slot_sems = [nc.alloc_semaphore(f"slot{d}") for d in range(1, N_CORES)]
    send_loc = nc.alloc_semaphore("send_loc")

    deferred = []  # (BassInstruction, sem, val) attached post-scheduling

    with tile.TileContext(nc) as tc:
        with tc.tile_pool(name="persist", bufs=1) as persist, \
             tc.tile_pool(name="state", bufs=2) as state, \
             tc.tile_pool(name="zbpool", bufs=3) as zbpool, \
             tc.tile_pool(name="astream", bufs=2) as astream, \
             tc.tile_pool(name="kpool", bufs=6) as kpool, \
             tc.tile_pool(name="work", bufs=2) as work, \
             tc.tile_pool(name="psum", bufs=1, space="PSUM") as psum:

            # ---- constants / persistent buffers -------------------------
            onec = persist.tile([128, 1], BF16, name="onec")
            nc.sync.dma_start(onec[:], ones_col[:])
            oner = persist.tile([1, NLOC], BF16, name="oner")
            nc.sync.dma_start(oner[:], ones_row[:])
            lsa = persist.tile([1, 1], F32, name="lsa")
            nc.sync.dma_start(lsa[:], log_inv_sa[:])
            cexp_c = persist.tile([128, 1], F32, name="cexp_c")
            nc.sync.dma_start(cexp_c[:], cexp0[:])

            # double-buffered receive buffers (slots 1..7)
            recvbuf = [persist.tile([128, 7 * PAYC], F8, name=f"recv{p}")
                       for p in (0, 1)]
            nc.sync.dma_start(recvbuf[0][:], recv0[:])

            # local state: fp32 master + fp8 payload (doubles as zb)
            zt = [state.tile([128, NLOC], F32, name=f"zt{ch}", tag=f"zt{ch}")
                  for ch in (0, 1)]
            for ch in (0, 1):
                nc.sync.dma_start(zt[ch][:], zt_local0[ch * 128:(ch + 1) * 128, :])
            zbp = zbpool.tile([128, PAYC], F8, name="zbp0t", tag="zbp")
            nc.sync.dma_start(zbp[:], zbp0[:])

            e_row = state.tile([1, NLOC], BF16, name="e_row", tag="e_row")
            nc.sync.dma_start(e_row[:], e_row0[:])

            for t in range(T):
                last = (t == T - 1)
                par = t % 2

                # ---- A_t: one 2 MB DMA into [128, 32*256] ---------------
                a_sb = astream.tile([128, NJB * D], F8, name=f"a_{t}", tag="a")
                nc.sync.dma_start(
                    a_sb[:].rearrange("p (j d) -> p j d", j=NJB),
                    a_b.ap()[t].rearrange("(j p) d -> p j d", p=128))

                aaff_sb = astream.tile([128, 2 * D], F8, name=f"aaff_{t}",
                                       tag="aaff")
                nc.sync.dma_start(
                    aaff_sb[:].rearrange("p (c d) -> p c d", c=2),
                    aaff_b.ap()[t].rearrange("(c p) d -> p c d", p=128))
                brow_t = astream.tile([1, D], BF16, name=f"brow_{t}", tag="brow")
                nc.sync.dma_start(brow_t[:], b_b.ap()[t, :, :])

                if not last:
                    zbp_nxt = zbpool.tile([128, PAYC], F8, name=f"zbp{t + 1}",
                                          tag="zbp")

                # ---- slot source views ----------------------------------
                def slot_z(d):
                    if d == 0:
                        return zbp[:, 0:1024]
                    return recvbuf[par][:, (d - 1) * PAYC:(d - 1) * PAYC + 1024]

                def slot_sq(d):
                    if d == 0:
                        return zbp[:, 1024:1032]
                    return recvbuf[par][:, (d - 1) * PAYC + 1024:
                                        (d - 1) * PAYC + 1032]

                # ---- bias: -c*sq_j per j-block, from slot sq bits -------
                # scalar operand = zbp const col (-CEXP): correct value AND
                # an ordering anchor (forces post-update emission on DVE).
                bias_d = []
                for d in range(N_CORES):
                    bt = work.tile([128, NJB_LOC], F32, name=f"bias_{t}_{d}",
                                   tag=f"bias{d}", bufs=2)
                    bi = nc.vector.tensor_scalar_mul(
                        bt[:], slot_sq(d).bitcast(BF16),
                        zbp[:, 1036:1040].bitcast(F32))
                    if t >= 1 and d >= 1:
                        deferred.append((bi, slot_sems[d - 1], 2 * t))
                    bias_d.append(bt)

                # ---- E broadcast: e_sb[p, i] = exp(-c*sq_i)/SA ----------
                e_ps = psum.tile([128, NLOC], F32, name=f"e_ps_{t}",
                                 tag="aux", bufs=2)
                nc.tensor.matmul(e_ps[:], oner[:, 0:128], e_row[:],
                                 start=True, stop=True)
                e_sb = work.tile([128, NLOC], F32, name=f"e_sb_{t}",
                                 tag="e_sb", bufs=2)
                nc.vector.tensor_copy(e_sb[:], e_ps[:])

                # ---- affine part: va[dh] = Aaff_t @ z_loc + b_t ---------
                va_sb = []
                aaff3 = aaff_sb[:].rearrange("k (r d) -> k r d", r=2)
                zb3 = zbp[:, 0:1024].rearrange("k (r i) -> k r i", r=2)
                for dh in (0, 1):
                    va = psum.tile([128, NLOC], F32, name=f"va_{t}_{dh}",
                                   tag="aux", bufs=2)
                    nc.tensor.matmul(va[:],
                                     aaff3[:, :, dh * 128:(dh + 1) * 128],
                                     zb3[:], start=True, stop=False,
                                     perf_mode=DR)
                    nc.tensor.matmul(va[:],
                                     brow_t[:, dh * 128:(dh + 1) * 128],
                                     oner[:], start=False, stop=True)
                    vs0 = work.tile([128, NLOC], F32, name=f"vs0_{t}_{dh}",
                                    tag=f"vs0{dh}", bufs=2)
                    nc.vector.tensor_scalar_mul(vs0[:], va[:], 1.0 / SAF)
                    vsb = work.tile([128, NLOC], F32, name=f"vasb_{t}_{dh}",
                                    tag=f"vasb{dh}", bufs=2)
                    nc.vector.tensor_add(vsb[:], vs0[:], zt[dh][:])
                    va_sb.append(vsb)

                # ---- main loop over block PAIRS (DoubleRow, fp8) --------
                vr = [psum.tile([128, NLOC], F32, name=f"vr_{t}_{dh}",
                                tag=f"vr{dh}", bufs=1) for dh in (0, 1)]
                k_pairs = [None] * NPAIR
                for qq in range(NPAIR + PIPE_LAG):
                    if qq < NPAIR:
                        q = qq
                        d = q // 2
                        z3 = slot_z(d).rearrange("k (r j) -> k r j", r=2)
                        k_p = kpool.tile([128, 2 * NLOC], F8,
                                         name=f"k_{t}_{q}", tag="k")
                        s_ps = psum.tile([128, 2 * NLOC], F32,
                                         name=f"s_{t}_{q}", tag="s", bufs=2)
                        for h in (0, 1):
                            jb = 2 * q + h
                            b = jb % NJB_LOC
                            mm = nc.tensor.matmul(
                                s_ps[:, h * NLOC:(h + 1) * NLOC],
                                z3[:, :, b * 128:(b + 1) * 128],
                                zb3[:], start=True, stop=True,
                                perf_mode=DR)
                            if t >= 1 and d >= 1:
                                deferred.append((mm, slot_sems[d - 1], 2 * t))
                            # K' = exp(2c*S - c*sq_j)
                            nc.scalar.activation(
                                k_p[:, h * NLOC:(h + 1) * NLOC],
                                s_ps[:, h * NLOC:(h + 1) * NLOC],
                                EXP, scale=2.0 * CEXP,
                                bias=bias_d[d][:, b:b + 1])
                        k_pairs[q] = k_p
                    if qq >= PIPE_LAG:
                        q = qq - PIPE_LAG
                        k_p = k_pairs[q]
                        a3 = a_sb[:, 2 * q * D:2 * (q + 1) * D].rearrange(
                            "k (r d) -> k r d", r=2)
                        k3 = k_p[:].rearrange("k (r i) -> k r i", r=2)
                        for dh in (0, 1):
                            mm = nc.tensor.matmul(
                                vr[dh][:],
                                a3[:, :, dh * 128:(dh + 1) * 128],
                                k3[:],
                                start=(q == 0),
                                stop=(q == NPAIR - 1 and dh == 1),
                                perf_mode=DR)


                # ---- update: z <- z + va + vr * E -----------------------
                zt_new = [state.tile([128, NLOC], F32, name=f"ztn_{t}_{ch}",
                                     tag=f"zt{ch}") for ch in (0, 1)]
                for dh in (0, 1):
                    t1 = work.tile([128, NLOC], F32, name=f"t1_{t}_{dh}",
                                   tag="t1", bufs=2)
                    nc.vector.tensor_mul(t1[:], vr[dh][:], e_sb[:])
                    nc.vector.tensor_add(zt_new[dh][:], t1[:],
                                         va_sb[dh][:])
                zt = zt_new

                if last:
                    for ch in (0, 1):
                        nc.sync.dma_start(
                            out_zt[ch * 128:(ch + 1) * 128, :], zt[ch][:])
                    break

                # ---- tail: build payload zbp(t+1) ------------------------
                z2 = [work.tile([128, NLOC], BF16, name=f"z2_{t}_{ch}",
                                tag=f"z2{ch}", bufs=2) for ch in (0, 1)]
                for ch in (0, 1):
                    nc.vector.tensor_copy(
                        zbp_nxt[:, ch * NLOC:(ch + 1) * NLOC], zt[ch][:])
                    nc.scalar.activation(z2[ch][:], zt[ch][:], SQUARE)

                # sq in column layout [128, 4] -> payload bf16 bits
                sqc_ps = psum.tile([128, NJB_LOC], F32, name=f"sqc_{t}",
                                   tag="aux", bufs=2)
                for ib in range(NJB_LOC):
                    for ch in (0, 1):
                        nc.tensor.matmul(sqc_ps[:, ib:ib + 1],
                                         z2[ch][:, ib * 128:(ib + 1) * 128],
                                         onec[:],
                                         start=(ch == 0), stop=(ch == 1))
                nc.vector.tensor_copy(
                    zbp_nxt[:, 1024:1032].bitcast(BF16), sqc_ps[:])
                nc.vector.tensor_copy(
                    zbp_nxt[:, 1036:1040].bitcast(F32), cexp_c[:])

                # sq in row layout -> e_row for next step
                sqr_ps = psum.tile([1, NLOC], F32, name=f"sqr_{t}",
                                   tag="aux", bufs=2)
                for ch in (0, 1):
                    nc.tensor.matmul(sqr_ps[:], onec[:], z2[ch][:],
                                     start=(ch == 0), stop=(ch == 1))
                e_row_new = state.tile([1, NLOC], BF16, name=f"er_{t}",
                                       tag="e_row")
                nc.scalar.activation(e_row_new[:], sqr_ps[:], EXP, scale=-CEXP,
                                     bias=lsa[:])
                e_row = e_row_new

                # ---- send descgen + fire the exchange -------------------
                # (preps must trace AFTER the zbp_nxt writes so the trigger
                # inherits the RAW dependency on the payload)
                for d in range(1, N_CORES):
                    rd = [None] * 8
                    rd[d] = _rdest(d)
                    nc.gpsimd.remote_dma_broadcast(
                        recvbuf[1 - par][:, (d - 1) * PAYC:d * PAYC],
                        zbp_nxt[:],
                        remote_sem=slot_sems[d - 1], local_sem=send_loc,
                        rdests=rd, queue_num=0)
                nc.gpsimd.trigger_dma(count=None, queue_num=0)

                zbp = zbp_nxt

    # Attach remote-arrival waits after Tile scheduling (the single-core
    # scheduling sim cannot see cross-core sem increments and would
    # deadlock). check=False: Tile may already have filled the wait slot.
    for bi, sem, val in deferred:
        bi.wait_op(sem, val, "sem-ge", check=False)

    nc.compile()
    return nc


def _prepare_in_maps(X, A, A_aff, b_aff):
    XT = np.ascontiguousarray(X.T.astype(np.float32))          # [D, N]
    XT8 = XT.astype(F8NP)
    sq0 = (X.astype(np.float32) ** 2).sum(axis=1)              # [N]
    a8 = (DT * SA * A.astype(np.float32)).astype(F8NP)         # [T, N, D]
    aaff_b = np.ascontiguousarray(
        (DT * SAF * A_aff.astype(np.float32)).transpose(0, 2, 1)).astype(F8NP)
    b_b = (DT * SAF * b_aff.astype(np.float32)).reshape(T, 1, D).astype(BF16NP)
    ones_col = np.ones((128, 1), dtype=BF16NP)
    ones_row = np.ones((1, NLOC), dtype=BF16NP)
    cexp0 = np.full((128, 1), -CEXP, dtype=np.float32)

    def payload(c):
        """[128, PAYC] fp8 payload for core c's rows."""
        p = np.zeros((128, PAYC), dtype=F8NP)
        cols = slice(c * NLOC, (c + 1) * NLOC)
        p[:, 0:512] = XT8[0:128, cols]
        p[:, 512:1024] = XT8[128:256, cols]
        sqc = np.ascontiguousarray(
            sq0[cols].reshape(NJB_LOC, 128).T).astype(BF16NP)  # [128, 4]
        p[:, 1024:1032] = sqc.view(F8NP)
        p[:, 1036:1040] = cexp0.view(F8NP)
        return p

    pays = [payload(c) for c in range(N_CORES)]

    in_maps = []
    for c in range(N_CORES):
        cols = slice(c * NLOC, (c + 1) * NLOC)
        recv0 = np.concatenate([pays[c ^ d] for d in range(1, N_CORES)],
                               axis=1)
        a_perm = np.concatenate(
            [a8[:, (c ^ d) * NLOC:((c ^ d) + 1) * NLOC, :]
             for d in range(N_CORES)], axis=1)
        in_maps.append({
            "zt_local0": np.ascontiguousarray(XT[:, cols]),
            "zbp0": pays[c],
            "recv0": np.ascontiguousarray(recv0),
            "e_row0": (np.exp(-CEXP * sq0[cols]) / SA)[None, :].astype(BF16NP),
            "a_b": np.ascontiguousarray(a_perm),
            "aaff_b": aaff_b,
            "b_b": b_b,
            "ones_col": ones_col,
            "ones_row": ones_row,
            "log_inv_sa": np.array([[np.log(1.0 / SA)]], dtype=np.float32),
            "cexp0": cexp0,
        })
    return in_maps


def _get_nc():
    if "nc" not in _CACHED:
        _CACHED["nc"] = _build()
    return _CACHED["nc"]


def kernel(X, A, A_aff, b_aff):
    X = np.asarray(X)
    A = np.asarray(A)
    A_aff = np.asarray(A_aff)
    b_aff = np.asarray(b_aff)
    nc = _get_nc()
    in_maps = _prepare_in_maps(X, A, A_aff, b_aff)
    res = bass_utils.run_bass_kernel_spmd(
        nc, in_maps, core_ids=list(range(N_CORES)))
    out = np.empty((N, D), dtype=np.float32)
    for c in range(N_CORES):
        out[c * NLOC:(c + 1) * NLOC, :] = res.results[c]["out_zt"].T
    return out


# revision 12
# speedup vs baseline: 18.4139x; 18.4139x over previous
"""Trainium2 Bass kernel for DiffeomorphicLearner (gnn_message_passing).

Math (per step t, T=8 steps):
    sq_i  = ||z_i||^2
    K_ij  = exp((2 z_i.z_j - sq_i - sq_j) / (2 rho^2))
    v     = Z @ Aaff_t.T + b_t + K @ A_t
    Z    <- Z + DT * v

Distribution: row-parallel over N=4096 across 8 cores (512 rows each).
The per-step exchange of Z is done with direct core-to-core SBUF writes
(remote_dma_broadcast) instead of a collective AllGather: each core
broadcasts its fp8 payload [z^T (2ch x 512) | sq bits | const] to its 7
peers, landing in XOR-slot order (slot d holds data from core me^d, so
the receive address is compile-time constant in the SPMD program).
Per-slot arrival semaphores let the next step's S = Z_j.Z_i matmuls
consume slots stale-first (self slot 0 first, same-die next, cross-die
last), hiding the wire latency behind compute. A data-less notify
broadcast implements the WAR handshake for the double-buffered receive
buffers. All remote-arrival waits are attached to the consumer
instructions AFTER Tile scheduling (the single-core scheduling sim
cannot see cross-core semaphore increments).

Cross-die D2D lanes route with bit 1 of the tpb flipped (measured:
a send addressed (0,d) with d&4 lands on core s^d^2) — compensated by
addressing (0, d^2) for those slots.

Precision: all matmuls fp8 DoubleRow / fp32 PSUM. exp(-c sq_j) enters
as a per-partition ACT bias (bias_col); exp(-c sq_i) post-multiplies
vr. The A_t stream is XOR-permuted per core on the host so slot order
matches.
"""

import numpy as np
import ml_dtypes

import concourse.bass as bass
import concourse.tile as tile
from concourse import bacc, mybir
from concourse import bass_utils

BF16NP = ml_dtypes.bfloat16
F8NP = ml_dtypes.float8_e4m3

N_CORES = 8
N, D, T = 4096, 256, 8
RHO = 16.0
DT = 1.0 / T
CEXP = 1.0 / (2.0 * RHO * RHO)  # 1/512
SA = 64.0                      # fp8 scale for A (values are subnormal otherwise)
SAF = 64.0                     # fp8 scale for A_aff / b_aff

NLOC = N // N_CORES            # 512 rows per core
NJB = N // 128                 # 32 j-blocks of 128
NJB_LOC = NLOC // 128          # 4 j-blocks per slot
NPAIR = NJB // 2               # 16 block pairs (DoubleRow)
PAYC = 1040                    # payload cols: 1024 z + 8 sq + 1 cexp + 7 pad
PIPE_LAG = 2                   # pair skew between S/exp and K@A matmuls

F32 = mybir.dt.float32
BF16 = mybir.dt.bfloat16
F8 = mybir.dt.float8e4
DR = mybir.MatmulPerfMode.DoubleRow

_CACHED = {}


def _rdest(d):
    """XOR-relative dest for slot d, compensating the D2D bit-1 flip."""
    return (0, d ^ 2) if d & 4 else (0, d)


def _build():
    nc = bacc.Bacc("TRN2", target_bir_lowering=False, debug=False,
                   num_devices=N_CORES, num_swdge_queues=1)

    # ---- DRAM I/O -------------------------------------------------------
    zt_local0 = nc.dram_tensor("zt_local0", [D, NLOC], F32, kind="ExternalInput")
    zbp0 = nc.dram_tensor("zbp0", [128, PAYC], F8, kind="ExternalInput")
    recv0 = nc.dram_tensor("recv0", [128, 7 * PAYC], F8, kind="ExternalInput")
    e_row0 = nc.dram_tensor("e_row0", [1, NLOC], BF16, kind="ExternalInput")
    a_b = nc.dram_tensor("a_b", [T, N, D], F8, kind="ExternalInput")
    aaff_b = nc.dram_tensor("aaff_b", [T, D, D], F8, kind="ExternalInput")
    b_b = nc.dram_tensor("b_b", [T, 1, D], BF16, kind="ExternalInput")
    ones_col = nc.dram_tensor("ones_col", [128, 1], BF16, kind="ExternalInput")
    ones_row = nc.dram_tensor("ones_row", [1, NLOC], BF16, kind="ExternalInput")
    log_inv_sa = nc.dram_tensor("log_inv_sa", [1, 1], F32, kind="ExternalInput")
    cexp0 = nc.dram_tensor("cexp0", [128, 1], F32, kind="ExternalInput")
    out_zt = nc.dram_tensor("out_zt", [D, NLOC], F32, kind="ExternalOutput")

    EXP = mybir.ActivationFunctionType.Exp
    SQUARE = mybir.ActivationFunctionType.Square

    # Semaphores for the remote exchange. No explicit WAR handshake is
    # needed for the double-buffered receive buffers: a peer can only fire
    # exchange(t+2) (which overwrites my parity-p buffer) after its own
    # step t+1, which consumes MY exchange(t+1) (slot-sem gated), which I
    # only send after update(t) -- transitively after all my parity-p
    # reads of step t. The slot sems provide the back-pressure.
    slot_sems = [nc.alloc_semaphore(f"slot{d}") for d in range(1, N_CORES)]
    send_loc = nc.alloc_semaphore("send_loc")

    deferred = []  # (BassInstruction, sem, val) attached post-scheduling

    with tile.TileContext(nc) as tc:
        with tc.tile_pool(name="persist", bufs=1) as persist, \
             tc.tile_pool(name="state", bufs=2) as state, \
             tc.tile_pool(name="zbpool", bufs=3) as zbpool, \
             tc.tile_pool(name="astream", bufs=2) as astream, \
             tc.tile_pool(name="kpool", bufs=6) as kpool, \
             tc.tile_pool(name="work", bufs=2) as work, \
             tc.tile_pool(name="psum", bufs=1, space="PSUM") as psum, \
             tc.tile_pool(name="dram", bufs=1, space="DRAM") as dram:

            # ---- constants / persistent buffers -------------------------
            onec = persist.tile([128, 1], BF16, name="onec")
            nc.sync.dma_start(onec[:], ones_col[:])
            oner = persist.tile([1, NLOC], BF16, name="oner")
            nc.sync.dma_start(oner[:], ones_row[:])
            lsa = persist.tile([1, 1], F32, name="lsa")
            nc.sync.dma_start(lsa[:], log_inv_sa[:])
            cexp_c = persist.tile([128, 1], F32, name="cexp_c")
            nc.sync.dma_start(cexp_c[:], cexp0[:])

            # rank-sync AllGather: the PJRT launch staggers core starts by
            # multiple ms; the first collective's entry barrier aligns them.
            # Nothing consumes the output; it overlaps step-0 compute.
            cc_in = dram.tile([1, 16], BF16, name="cc_in", bufs=1)
            cc_out = dram.tile([N_CORES, 16], BF16, name="cc_out", bufs=1,
                               addr_space="Shared")
            nc.sync.dma_start(cc_in[:], ones_row[:, 0:16])
            nc.gpsimd.collective_compute(
                "AllGather", mybir.AluOpType.bypass,
                replica_groups=[list(range(N_CORES))],
                ins=[cc_in[:].opt()], outs=[cc_out[:].opt()],
            )

            # double-buffered receive buffers (slots 1..7)
            recvbuf = [persist.tile([128, 7 * PAYC], F8, name=f"recv{p}")
                       for p in (0, 1)]
            nc.sync.dma_start(recvbuf[0][:], recv0[:])

            # local state: fp32 master + fp8 payload (doubles as zb)
            zt = [state.tile([128, NLOC], F32, name=f"zt{ch}", tag=f"zt{ch}")
                  for ch in (0, 1)]
            for ch in (0, 1):
                nc.sync.dma_start(zt[ch][:], zt_local0[ch * 128:(ch + 1) * 128, :])
            zbp = zbpool.tile([128, PAYC], F8, name="zbp0t", tag="zbp")
            nc.sync.dma_start(zbp[:], zbp0[:])

            e_row = state.tile([1, NLOC], BF16, name="e_row", tag="e_row")
            nc.sync.dma_start(e_row[:], e_row0[:])

            for t in range(T):
                last = (t == T - 1)
                par = t % 2

                # ---- A_t: one 2 MB DMA into [128, 32*256] ---------------
                a_sb = astream.tile([128, NJB * D], F8, name=f"a_{t}", tag="a")
                nc.sync.dma_start(
                    a_sb[:].rearrange("p (j d) -> p j d", j=NJB),
                    a_b.ap()[t].rearrange("(j p) d -> p j d", p=128))

                aaff_sb = astream.tile([128, 2 * D], F8, name=f"aaff_{t}",
                                       tag="aaff")
                nc.sync.dma_start(
                    aaff_sb[:].rearrange("p (c d) -> p c d", c=2),
                    aaff_b.ap()[t].rearrange("(c p) d -> p c d", p=128))
                brow_t = astream.tile([1, D], BF16, name=f"brow_{t}", tag="brow")
                nc.sync.dma_start(brow_t[:], b_b.ap()[t, :, :])

                if not last:
                    zbp_nxt = zbpool.tile([128, PAYC], F8, name=f"zbp{t + 1}",
                                          tag="zbp")

                # ---- slot source views ----------------------------------
                def slot_z(d):
                    if d == 0:
                        return zbp[:, 0:1024]
                    return recvbuf[par][:, (d - 1) * PAYC:(d - 1) * PAYC + 1024]

                def slot_sq(d):
                    if d == 0:
                        return zbp[:, 1024:1032]
                    return recvbuf[par][:, (d - 1) * PAYC + 1024:
                                        (d - 1) * PAYC + 1032]

                # ---- bias: -c*sq_j per j-block, from slot sq bits -------
                # scalar operand = zbp const col (-CEXP): correct value AND
                # an ordering anchor (forces post-update emission on DVE).
                bias_d = []
                for d in range(N_CORES):
                    bt = work.tile([128, NJB_LOC], F32, name=f"bias_{t}_{d}",
                                   tag=f"bias{d}", bufs=2)
                    bi = nc.vector.tensor_scalar_mul(
                        bt[:], slot_sq(d).bitcast(BF16),
                        zbp[:, 1036:1040].bitcast(F32))
                    if t >= 1 and d >= 1:
                        deferred.append((bi, slot_sems[d - 1], 2 * t))
                    bias_d.append(bt)

                # ---- E broadcast: e_sb[p, i] = exp(-c*sq_i)/SA ----------
                e_ps = psum.tile([128, NLOC], F32, name=f"e_ps_{t}",
                                 tag="aux", bufs=2)
                nc.tensor.matmul(e_ps[:], oner[:, 0:128], e_row[:],
                                 start=True, stop=True)
                e_sb = work.tile([128, NLOC], F32, name=f"e_sb_{t}",
                                 tag="e_sb", bufs=2)
                nc.vector.tensor_copy(e_sb[:], e_ps[:])

                # ---- affine part: va[dh] = Aaff_t @ z_loc + b_t ---------
                va_sb = []
                aaff3 = aaff_sb[:].rearrange("k (r d) -> k r d", r=2)
                zb3 = zbp[:, 0:1024].rearrange("k (r i) -> k r i", r=2)
                for dh in (0, 1):
                    va = psum.tile([128, NLOC], F32, name=f"va_{t}_{dh}",
                                   tag="aux", bufs=2)
                    nc.tensor.matmul(va[:],
                                     aaff3[:, :, dh * 128:(dh + 1) * 128],
                                     zb3[:], start=True, stop=False,
                                     perf_mode=DR)
                    nc.tensor.matmul(va[:],
                                     brow_t[:, dh * 128:(dh + 1) * 128],
                                     oner[:], start=False, stop=True)
                    vs0 = work.tile([128, NLOC], F32, name=f"vs0_{t}_{dh}",
                                    tag=f"vs0{dh}", bufs=2)
                    nc.vector.tensor_scalar_mul(vs0[:], va[:], 1.0 / SAF)
                    vsb = work.tile([128, NLOC], F32, name=f"vasb_{t}_{dh}",
                                    tag=f"vasb{dh}", bufs=2)
                    nc.vector.tensor_add(vsb[:], vs0[:], zt[dh][:])
                    va_sb.append(vsb)

                # ---- main loop over block PAIRS (DoubleRow, fp8) --------
                vr = [psum.tile([128, NLOC], F32, name=f"vr_{t}_{dh}",
                                tag=f"vr{dh}", bufs=1) for dh in (0, 1)]
                k_pairs = [None] * NPAIR
                for qq in range(NPAIR + PIPE_LAG):
                    if qq < NPAIR:
                        q = qq
                        d = q // 2
                        z3 = slot_z(d).rearrange("k (r j) -> k r j", r=2)
                        k_p = kpool.tile([128, 2 * NLOC], F8,
                                         name=f"k_{t}_{q}", tag="k")
                        s_ps = psum.tile([128, 2 * NLOC], F32,
                                         name=f"s_{t}_{q}", tag="s", bufs=2)
                        for h in (0, 1):
                            jb = 2 * q + h
                            b = jb % NJB_LOC
                            mm = nc.tensor.matmul(
                                s_ps[:, h * NLOC:(h + 1) * NLOC],
                                z3[:, :, b * 128:(b + 1) * 128],
                                zb3[:], start=True, stop=True,
                                perf_mode=DR)
                            if t >= 1 and d >= 1:
                                deferred.append((mm, slot_sems[d - 1], 2 * t))
                            # K' = exp(2c*S - c*sq_j)
                            nc.scalar.activation(
                                k_p[:, h * NLOC:(h + 1) * NLOC],
                                s_ps[:, h * NLOC:(h + 1) * NLOC],
                                EXP, scale=2.0 * CEXP,
                                bias=bias_d[d][:, b:b + 1])
                        k_pairs[q] = k_p
                    if qq >= PIPE_LAG:
                        q = qq - PIPE_LAG
                        k_p = k_pairs[q]
                        a3 = a_sb[:, 2 * q * D:2 * (q + 1) * D].rearrange(
                            "k (r d) -> k r d", r=2)
                        k3 = k_p[:].rearrange("k (r i) -> k r i", r=2)
                        for dh in (0, 1):
                            mm = nc.tensor.matmul(
                                vr[dh][:],
                                a3[:, :, dh * 128:(dh + 1) * 128],
                                k3[:],
                                start=(q == 0),
                                stop=(q == NPAIR - 1 and dh == 1),
                                perf_mode=DR)


                # ---- update: z <- z + va + vr * E -----------------------
                zt_new = [state.tile([128, NLOC], F32, name=f"ztn_{t}_{ch}",
                                     tag=f"zt{ch}") for ch in (0, 1)]
                for dh in (0, 1):
                    t1 = work.tile([128, NLOC], F32, name=f"t1_{t}_{dh}",
                                   tag="t1", bufs=2)
                    nc.vector.tensor_mul(t1[:], vr[dh][:], e_sb[:])
                    nc.vector.tensor_add(zt_new[dh][:], t1[:],
                                         va_sb[dh][:])
                zt = zt_new

                if last:
                    for ch in (0, 1):
                        nc.sync.dma_start(
                            out_zt[ch * 128:(ch + 1) * 128, :], zt[ch][:])
                    break

                # ---- tail: build payload zbp(t+1) ------------------------
                z2 = [work.tile([128, NLOC], BF16, name=f"z2_{t}_{ch}",
                                tag=f"z2{ch}", bufs=2) for ch in (0, 1)]
                for ch in (0, 1):
                    nc.vector.tensor_copy(
                        zbp_nxt[:, ch * NLOC:(ch + 1) * NLOC], zt[ch][:])
                    nc.scalar.activation(z2[ch][:], zt[ch][:], SQUARE)

                # sq in column layout [128, 4] -> payload bf16 bits
                sqc_ps = psum.tile([128, NJB_LOC], F32, name=f"sqc_{t}",
                                   tag="aux", bufs=2)
                for ib in range(NJB_LOC):
                    for ch in (0, 1):
                        nc.tensor.matmul(sqc_ps[:, ib:ib + 1],
                                         z2[ch][:, ib * 128:(ib + 1) * 128],
                                         onec[:],
                                         start=(ch == 0), stop=(ch == 1))
                nc.vector.tensor_copy(
                    zbp_nxt[:, 1024:1032].bitcast(BF16), sqc_ps[:])
                nc.vector.tensor_copy(
                    zbp_nxt[:, 1036:1040].bitcast(F32), cexp_c[:])

                # sq in row layout -> e_row for next step
                sqr_ps = psum.tile([1, NLOC], F32, name=f"sqr_{t}",
                                   tag="aux", bufs=2)
                for ch in (0, 1):
                    nc.tensor.matmul(sqr_ps[:], onec[:], z2[ch][:],
                                     start=(ch == 0), stop=(ch == 1))
                e_row_new = state.tile([1, NLOC], BF16, name=f"er_{t}",
                                       tag="e_row")
                nc.scalar.activation(e_row_new[:], sqr_ps[:], EXP, scale=-CEXP,
                                     bias=lsa[:])
                e_row = e_row_new

                # ---- send descgen + fire the exchange -------------------
                # (preps must trace AFTER the zbp_nxt writes so the trigger
                # inherits the RAW dependency on the payload)
                for d in range(1, N_CORES):
                    rd = [None] * 8
                    rd[d] = _rdest(d)
                    nc.gpsimd.remote_dma_broadcast(
                        recvbuf[1 - par][:, (d - 1) * PAYC:d * PAYC],
                        zbp_nxt[:],
                        remote_sem=slot_sems[d - 1], local_sem=send_loc,
                        rdests=rd, queue_num=0)
                nc.gpsimd.trigger_dma(count=None, queue_num=0)

                zbp = zbp_nxt

    # Attach remote-arrival waits after Tile scheduling (the single-core
    # scheduling sim cannot see cross-core sem increments and would
    # deadlock). check=False: Tile may already have filled the wait slot.
    for bi, sem, val in deferred:
        bi.wait_op(sem, val, "sem-ge", check=False)

    nc.compile()
    return nc


def _prepare_in_maps(X, A, A_aff, b_aff):
    XT = np.ascontiguousarray(X.T.astype(np.float32))          # [D, N]
    XT8 = XT.astype(F8NP)
    sq0 = (X.astype(np.float32) ** 2).sum(axis=1)              # [N]
    a8 = (DT * SA * A.astype(np.float32)).astype(F8NP)         # [T, N, D]
    aaff_b = np.ascontiguousarray(
        (DT * SAF * A_aff.astype(np.float32)).transpose(0, 2, 1)).astype(F8NP)
    b_b = (DT * SAF * b_aff.astype(np.float32)).reshape(T, 1, D).astype(BF16NP)
    ones_col = np.ones((128, 1), dtype=BF16NP)
    ones_row = np.ones((1, NLOC), dtype=BF16NP)
    cexp0 = np.full((128, 1), -CEXP, dtype=np.float32)

    def payload(c):
        """[128, PAYC] fp8 payload for core c's rows."""
        p = np.zeros((128, PAYC), dtype=F8NP)
        cols = slice(c * NLOC, (c + 1) * NLOC)
        p[:, 0:512] = XT8[0:128, cols]
        p[:, 512:1024] = XT8[128:256, cols]
        sqc = np.ascontiguousarray(
            sq0[cols].reshape(NJB_LOC, 128).T).astype(BF16NP)  # [128, 4]
        p[:, 1024:1032] = sqc.view(F8NP)
        p[:, 1036:1040] = cexp0.view(F8NP)
        return p

    pays = [payload(c) for c in range(N_CORES)]

    in_maps = []
    for c in range(N_CORES):
        cols = slice(c * NLOC, (c + 1) * NLOC)
        recv0 = np.concatenate([pays[c ^ d] for d in range(1, N_CORES)],
                               axis=1)
        a_perm = np.concatenate(
            [a8[:, (c ^ d) * NLOC:((c ^ d) + 1) * NLOC, :]
             for d in range(N_CORES)], axis=1)
        in_maps.append({
            "zt_local0": np.ascontiguousarray(XT[:, cols]),
            "zbp0": pays[c],
            "recv0": np.ascontiguousarray(recv0),
            "e_row0": (np.exp(-CEXP * sq0[cols]) / SA)[None, :].astype(BF16NP),
            "a_b": np.ascontiguousarray(a_perm),
            "aaff_b": aaff_b,
            "b_b": b_b,
            "ones_col": ones_col,
            "ones_row": ones_row,
            "log_inv_sa": np.array([[np.log(1.0 / SA)]], dtype=np.float32),
            "cexp0": cexp0,
        })
    return in_maps


def _get_nc():
    if "nc" not in _CACHED:
        _CACHED["nc"] = _build()
    return _CACHED["nc"]


def kernel(X, A, A_aff, b_aff):
    X = np.asarray(X)
    A = np.asarray(A)
    A_aff = np.asarray(A_aff)
    b_aff = np.asarray(b_aff)
    nc = _get_nc()
    in_maps = _prepare_in_maps(X, A, A_aff, b_aff)
    res = bass_utils.run_bass_kernel_spmd(
        nc, in_maps, core_ids=list(range(N_CORES)))
    out = np.empty((N, D), dtype=np.float32)
    for c in range(N_CORES):
        out[c * NLOC:(c + 1) * NLOC, :] = res.results[c]["out_zt"].T
    return out
